# revision 24
# baseline (speedup 1.0000x reference)
"""Trainium2 Bass kernel for nn_EpisodicMemory (trail_read_all, eval, 2 steps).

Sharding: data-parallel over BS — one batch-sample per NeuronCore (8 cores).
Per-bank params (tau/alpha/bias) are baked in as immediates at trace time.

Layout strategy: scores are computed TRANSPOSED (scoresT[m,n] = K @ y^T) so
exp() lands directly in the U^T[m,n] layout the dU matmul wants as lhsT — no
per-tile U transpose exists anywhere.  y is maintained in both natural [n,d]
(f16) and transposed [d,n] (f16) form; y^T blocks come from PE transposes
(f16 = 1 cyc/row) staged in a borrowed dU PSUM bank, copied out by Act.

Per n-chunk of 512 (b = bank, t = step):
    scoresT = K_b @ y^T           f16 matmuls, PSUM [m, n]
    U^T     = exp(scoresT/tau)    Act, PSUM -> SBUF bf16 (bf16 for exp range)
    dU      = U^T.T @ V_b         bf16 matmuls, PSUM [n, d]
    Z       = U^T.T @ ones        tiny bf16 matmuls -> scoresT bank cols 0:4
    dots_n  = sum_d y*dU          DVE STT accum from PSUM
    g       = sigmoid(alpha*dots*rz/D + bias)*rz   (rz = 1/Z)
    gdU     = g*dU                Act scale-copy (j<2) / DVE STT (j>=2)
    acc    += gdU ; y' = y + gdU  Pool adds (j<2) / DVE STT (j>=2)
    y'^T    = PE transpose + Act copy, t=0 only

f16 for the score-side operands (K, y, yT) keeps the cancellation-sensitive
inner products at 10-bit mantissa; U/V are bf16 (exp needs bf16 range; value
noise there is benign).  PSUM accumulation is always f32; acc stays f32.
PSUM: scoresT 2 slots x 2 banks (two-chunk lookahead), dU/transpose pool
4 x 1 bank.  Engine notes baked in from ISA probing: Pool (gpsimd) cannot
touch PSUM and has no TensorScalarPtr/divide; tensor_tensor_reduce faults on
HW; f32r matmuls reject free-dim < 256 (hence bf16 U/ones for the Z sums).
"""

import os

import numpy as np

import concourse.bass as bass
import concourse.mybir as mybir
import concourse.tile as tile
from concourse import bacc
from concourse.bass_utils import run_bass_kernel_spmd

dt = mybir.dt
AL = mybir.AluOpType
AF = mybir.ActivationFunctionType

BS, B, M, D, N = 8, 4, 256, 256, 2048
P = 128
CW = 512          # n-chunk width processed per pipeline step
NCH = N // CW     # 4 chunks
NT = N // P       # 16 n-tiles
N_STEPS = 2

f32 = dt.float32
f16 = dt.float16
bf16 = dt.bfloat16


def _build(variant: str, tau, alpha, bias, use_mask: bool, reps: int = 1):
    del variant

    nc = bacc.Bacc(None, target_bir_lowering=False)
    seed_d = nc.dram_tensor("seed", [N, D], f32, kind="ExternalInput")
    emk_d = nc.dram_tensor("em_K", [B, M, D], f32, kind="ExternalInput")
    emv_d = nc.dram_tensor("em_V", [B, M, D], f32, kind="ExternalInput")
    out_d = nc.dram_tensor("out", [N, D], f32, kind="ExternalOutput")
    if use_mask:
        # ln(mask) per bank / m-chunk: [B, 128, 2]; -1e30 where inactive
        msk_d = nc.dram_tensor("lnmask", [B, P, 2], f32, kind="ExternalInput")

    with tile.TileContext(nc) as tc:
        import contextlib

        ctx = contextlib.ExitStack()
        with ctx:
            pool = lambda name, bufs, space="SBUF": ctx.enter_context(
                tc.tile_pool(name=name, bufs=bufs, space=space)
            )
            p_s = pool("p_s", NT)          # seed natural [P, D] f16
            p_sT = pool("p_sT", 1)         # seed transposed [P, 2N] f16
            p_y1T = pool("p_y1T", 2)       # y1 transposed [P, 2N] f16
            p_kT = pool("p_kT", B)         # K^T [P, 2M] f16
            p_v = pool("p_v", 2 * B)       # V chunks [P, D] bf16
            p_y1 = pool("p_y1", 2 * NT)    # y1 natural [P, D] f16
            p_acc = pool("p_acc", NT)      # output accumulator [P, D] f32
            p_UT = pool("p_UT", 4)         # exp(scoresT) [P, 2*CW] bf16
            p_scr = pool("p_scr", 14)      # ttr product dump / gdU [P, D]
            p_tiny = pool("p_tiny", 64)    # gate vectors [P, 4] f32
            p_stage = pool("p_stage", 10)  # f32 staging [P, D]
            p_const = pool("p_const", 1)
            p_msk = pool("p_msk", B) if use_mask else None
            p_sc = pool("p_sc", 2, space="PSUM")   # scoresT [P, 1024] (2 banks each)
            p_du = pool("p_du", 4, space="PSUM")   # dU / transpose staging (1 bank each)

            ones_f = p_const.tile([P, 1], f32, name="ones_f")
            nc.vector.memset(ones_f, 1.0)
            ones = p_const.tile([P, 1], bf16, name="ones")
            nc.vector.tensor_copy(ones, ones_f)
            from concourse.masks import make_identity
            identf = p_const.tile([P, P], f32, name="identf")
            make_identity(nc, identf)
            ident = p_const.tile([P, P], f16, name="ident")
            nc.vector.tensor_copy(ident, identf)

            for rep in range(reps):
                # ---------------- preload ----------------
                # seed: stage f32, convert to f16, PE-transpose into sT
                sT = p_sT.tile([P, 2 * N], f16, name="sT")
                sb_s = []
                for i in range(NT):
                    st_i = p_stage.tile([P, D], f32, name="st_i")
                    nc.sync.dma_start(st_i, seed_d[i * P : (i + 1) * P, :])
                    s_i = p_s.tile([P, D], f16, name="s_i")
                    eng = [nc.vector, nc.gpsimd, nc.scalar][i % 3]
                    if eng is nc.scalar:
                        nc.scalar.copy(s_i, st_i)
                    else:
                        eng.tensor_copy(s_i, st_i)
                    sb_s.append(s_i)
                for g4 in range(4):
                    pts = p_du.tile([P, 2 * CW], f16, name="pts", tag="du")
                    for jj in range(4):
                        i = g4 * 4 + jj
                        for c in range(2):
                            nc.tensor.transpose(
                                pts[:, c * CW + jj * P : c * CW + (jj + 1) * P],
                                sb_s[i][:, c * P : (c + 1) * P],
                                ident,
                            )
                    for c in range(2):
                        dst = sT[:, c * N + g4 * CW : c * N + (g4 + 1) * CW]
                        srcp = pts[:, c * CW : (c + 1) * CW]
                        if (2 * g4 + c) % 2 == 0:
                            nc.vector.tensor_copy(dst, srcp)
                        else:
                            nc.scalar.copy(dst, srcp)

                # K^T per bank: [P, 2M]; cols c*M+mt*P <- transpose of K chunk
                kT = []
                for b in range(B):
                    ek16 = []
                    for mt in range(2):
                        ek_t = p_stage.tile([P, D], f32, name="ek_t")
                        nc.sync.dma_start(ek_t, emk_d[b, mt * P : (mt + 1) * P, :])
                        ek16_t = p_stage.tile([P, D], f16, name="ek16", tag="ek16")
                        eng = [nc.vector, nc.gpsimd, nc.scalar][(2 * b + mt) % 3]
                        if eng is nc.scalar:
                            nc.scalar.copy(ek16_t, ek_t)
                        else:
                            eng.tensor_copy(ek16_t, ek_t)
                        ek16.append(ek16_t)
                    ptk = p_du.tile([P, 2 * M], f16, name="ptk", tag="du")
                    for c in range(2):
                        for mt in range(2):
                            nc.tensor.transpose(
                                ptk[:, c * M + mt * P : c * M + (mt + 1) * P],
                                ek16[mt][:, c * P : (c + 1) * P],
                                ident,
                            )
                    kT_b = p_kT.tile([P, 2 * M], f16, name="kT_b")
                    if b % 2 == 0:
                        nc.vector.tensor_copy(kT_b, ptk)
                    else:
                        nc.scalar.copy(kT_b, ptk)
                    kT.append(kT_b)

                # V chunks bf16 (matmul partner of bf16 U^T)
                v = []
                for b in range(B):
                    v_b = []
                    for c in range(2):
                        ev_t = p_stage.tile([P, D], f32, name="ev_t")
                        nc.sync.dma_start(ev_t, emv_d[b, c * P : (c + 1) * P, :])
                        v_bc = p_v.tile([P, D], bf16, name="v_bc")
                        eng = [nc.vector, nc.gpsimd, nc.scalar][(2 * b + c) % 3]
                        if eng is nc.scalar:
                            nc.scalar.copy(v_bc, ev_t)
                        else:
                            eng.tensor_copy(v_bc, ev_t)
                        v_b.append(v_bc)
                    v.append(v_b)

                msk = []
                if use_mask:
                    for b in range(B):
                        m_b = p_msk.tile([P, 2], f32, name="m_b")
                        nc.sync.dma_start(m_b, msk_d[b])
                        msk.append(m_b)

                # ---------------- main loop ----------------
                jobs = [(b, t, k) for b in range(B) for t in range(2) for k in range(NCH)]
                acc = [None] * NT
                y1_nat = [[None] * NT for _ in range(B)]
                y1T = [None] * B

                def emit_sc(job):
                    b, t, k = job
                    sc = p_sc.tile([P, 2 * CW], f32, name="sc", tag="sc")
                    srcT = sT if t == 0 else y1T[b]
                    for mt in range(2):
                        for c in range(2):
                            nc.tensor.matmul(
                                sc[:, mt * CW : (mt + 1) * CW],
                                kT[b][:, c * M + mt * P : c * M + (mt + 1) * P],
                                srcT[:, c * N + k * CW : c * N + (k + 1) * CW],
                                start=(c == 0),
                                stop=(c == 1),
                            )
                    return sc

                def emit_exp(job, sc):
                    b, t, k = job
                    UT = p_UT.tile([P, 2 * CW], bf16, name="UT")
                    if use_mask:
                        for mt in range(2):
                            nc.scalar.activation(
                                UT[:, mt * CW : (mt + 1) * CW],
                                sc[:, mt * CW : (mt + 1) * CW],
                                AF.Exp,
                                scale=1.0 / tau[b],
                                bias=msk[b][:, mt : mt + 1],
                            )
                    else:
                        nc.scalar.activation(UT, sc, AF.Exp, scale=1.0 / tau[b])
                    return UT

                def emit_duz(job, sc, UT):
                    dus = []
                    for h in range(2):
                        du_h = p_du.tile([P, 2 * D], f32, name="du_h", tag="du")
                        dus.append(du_h)
                    for j in range(4):
                        sl = dus[j // 2][:, (j % 2) * D : (j % 2 + 1) * D]
                        for mt in range(2):
                            nc.tensor.matmul(
                                sl,
                                UT[:, mt * CW + j * P : mt * CW + (j + 1) * P],
                                v[job[0]][mt],
                                start=(mt == 0),
                                stop=(mt == 1),
                            )
                    for j in range(4):
                        for mt in range(2):
                            nc.tensor.matmul(
                                sc[:, j : j + 1],
                                UT[:, mt * CW + j * P : mt * CW + (j + 1) * P],
                                ones,
                                start=(mt == 0),
                                stop=(mt == 1),
                                skip_group_check=True,
                            )
                    return dus

                def du_slice(dus, j):
                    return dus[j // 2][:, (j % 2) * D : (j % 2 + 1) * D]

                def emit_rz(job, sc):
                    rzs = p_tiny.tile([P, 4], f32, name="rzs")
                    nc.vector.reciprocal(rzs, sc[:, 0:4])
                    return rzs

                def emit_dot(job, dus, ynat):
                    b, t, k = job
                    dots = p_tiny.tile([P, 4], f32, name="dots")
                    for j in range(4):
                        scr = p_scr.tile([P, D], bf16, name="scr")
                        nc.vector.scalar_tensor_tensor(
                            scr,
                            du_slice(dus, j),
                            1.0,
                            ynat[k * 4 + j],
                            AL.bypass,
                            AL.mult,
                            accum_out=dots[:, j : j + 1],
                        )
                    return dots

                def emit_dn(job, rzs, dots):
                    dn = p_tiny.tile([P, 4], f32, name="dn")
                    nc.vector.tensor_tensor(dn, dots, rzs, AL.mult)
                    return dn

                def emit_e1s1(job, dn):
                    b, t, k = job
                    e1 = p_tiny.tile([P, 4], f32, name="e1")
                    nc.scalar.activation(
                        e1, dn, AF.Exp, scale=-alpha[b] / D, bias=-bias[b]
                    )
                    s1 = p_tiny.tile([P, 4], f32, name="s1")
                    nc.scalar.add(s1, e1, 1.0)
                    return s1

                def emit_gB(job, s1, rzs):
                    gt = p_tiny.tile([P, 4], f32, name="gt")
                    nc.vector.reciprocal(gt, s1)
                    g = p_tiny.tile([P, 4], f32, name="g")
                    nc.vector.tensor_tensor(g, gt, rzs, AL.mult)
                    return g

                def emit_upd(job, dus, g, ynat):
                    b, t, k = job
                    for j in range(4):
                        i = k * 4 + j
                        gj = g[:, j : j + 1]
                        sl = du_slice(dus, j)
                        if j < 2:
                            # Act scale-copy makes gdU; Pool does the adds
                            gdu = p_scr.tile([P, D], bf16, name="gdu", tag="gdu")
                            nc.scalar.mul(gdu, sl, gj)
                            if b == 0 and t == 0:
                                a_i = p_acc.tile([P, D], f32, name="a_i")
                                nc.gpsimd.tensor_copy(a_i, gdu)
                                acc[i] = a_i
                            else:
                                nc.gpsimd.tensor_tensor(acc[i], gdu, acc[i], AL.add)
                            if t == 0:
                                y1_i = p_y1.tile([P, D], f16, name="y1_i")
                                nc.gpsimd.tensor_tensor(y1_i, gdu, ynat[i], AL.add)
                                y1_nat[b][i] = y1_i
                        else:
                            if b == 0 and t == 0:
                                a_i = p_acc.tile([P, D], f32, name="a_i")
                                nc.vector.tensor_scalar(a_i, sl, gj, None, AL.mult)
                                acc[i] = a_i
                            else:
                                nc.vector.scalar_tensor_tensor(
                                    acc[i], sl, gj, acc[i], AL.mult, AL.add
                                )
                            if t == 0:
                                y1_i = p_y1.tile([P, D], f16, name="y1_i")
                                nc.vector.scalar_tensor_tensor(
                                    y1_i, sl, gj, ynat[i], AL.mult, AL.add
                                )
                                y1_nat[b][i] = y1_i
                        if b == B - 1 and t == 1:
                            nc.sync.dma_start(
                                out_d[i * P : (i + 1) * P, :], acc[i]
                            )

                def emit_tr(job):
                    b, t, k = job
                    if y1T[b] is None:
                        y1T[b] = p_y1T.tile([P, 2 * N], f16, name="y1T_b")
                    pt = p_du.tile([P, 2 * CW], f16, name="pt", tag="du")
                    for j in range(4):
                        i = k * 4 + j
                        for c in range(2):
                            nc.tensor.transpose(
                                pt[:, c * CW + j * P : c * CW + (j + 1) * P],
                                y1_nat[b][i][:, c * P : (c + 1) * P],
                                ident,
                            )
                    nc.scalar.copy(
                        y1T[b][:, 0 * N + k * CW : 0 * N + (k + 1) * CW],
                        pt[:, 0:CW],
                    )
                    nc.scalar.copy(
                        y1T[b][:, 1 * N + k * CW : 1 * N + (k + 1) * CW],
                        pt[:, CW : 2 * CW],
                    )

                # Software pipeline: sc has a two-chunk lookahead, exp one
                # chunk, and the gate tail (sigmoid reciprocal + updates) is
                # deferred one body so DVE runs chunk k+1's dots while Act
                # computes chunk k's sigmoid — no head-of-line stall on DVE.
                scs = {0: emit_sc(jobs[0])}
                UTs = {0: emit_exp(jobs[0], scs[0])}
                scs[1] = emit_sc(jobs[1])
                pending_tr = None
                for idx, job in enumerate(jobs):
                    b, t, k = job
                    ynat = sb_s if t == 0 else y1_nat[b]
                    sc_cur = scs.pop(idx)
                    dus = emit_duz(job, sc_cur, UTs.pop(idx))
                    rzs = emit_rz(job, sc_cur)
                    dots = emit_dot(job, dus, ynat)
                    if idx + 2 < len(jobs):
                        scs[idx + 2] = emit_sc(jobs[idx + 2])
                    if idx + 1 < len(jobs):
                        UTs[idx + 1] = emit_exp(jobs[idx + 1], scs[idx + 1])
                    dn = emit_dn(job, rzs, dots)
                    s1 = emit_e1s1(job, dn)
                    g = emit_gB(job, s1, rzs)
                    if pending_tr is not None:
                        emit_tr(pending_tr)
                        pending_tr = None
                    emit_upd(job, dus, g, ynat)
                    if t == 0:
                        pending_tr = job
                if pending_tr is not None:
                    emit_tr(pending_tr)

    nc.compile()
    return nc


def kernel(**inputs):
    seed = np.ascontiguousarray(np.asarray(inputs["seed"], dtype=np.float32))
    em_K = np.ascontiguousarray(np.asarray(inputs["em_K"], dtype=np.float32))
    em_V = np.ascontiguousarray(np.asarray(inputs["em_V"], dtype=np.float32))
    em_S = np.asarray(inputs["em_S"], dtype=np.float32)
    gate_alpha = np.asarray(inputs["gate_alpha"], dtype=np.float32)
    gate_bias = np.asarray(inputs["gate_bias"], dtype=np.float32)
    raw_tau = np.asarray(inputs["raw_tau"], dtype=np.float32)

    variant = os.environ.get("EM_VARIANT", "f32r")
    tau = [float(np.log1p(np.exp(raw_tau[b])) + 0.1) for b in range(B)]
    alpha = [float(gate_alpha[b]) for b in range(B)]
    bias = [float(gate_bias[b]) for b in range(B)]
    use_mask = bool((em_S <= 0).any())

    nc = _build(variant, tau, alpha, bias, use_mask)

    in_maps = []
    for c in range(BS):
        m = {"seed": seed[c], "em_K": em_K[c], "em_V": em_V[c]}
        if use_mask:
            mask = (em_S[c] > 0)  # [B, M]
            lnm = np.where(mask, 0.0, -1e30).astype(np.float32)
            m["lnmask"] = np.ascontiguousarray(
                lnm.reshape(B, 2, P).transpose(0, 2, 1)
            )
        in_maps.append(m)

    res = run_bass_kernel_spmd(nc, in_maps, core_ids=list(range(BS)))
    out = np.stack([res.results[c]["out"] for c in range(BS)], axis=0)
    return out.astype(np.float32)
